# revision 24
# baseline (speedup 1.0000x reference)
"""Channel-attention kernel for Trainium2 (Bass/Tile), 8-core data parallel.

Computes, per batch sample b (x: [B=8, H=128, W=128, C=256] fp32):
    a   = x[b].reshape(N=16384, C)
    G   = a^T @ a                      # [C, C]
    att = softmax(G, axis=-1)
    out = x[b] + beta * (a @ att^T)

Sharding: pure data parallel, one sample per NeuronCore (B == n_cores == 8).

Per-core pipeline (N=16384 rows split into 128 chunks of 128):
  Phase 1: stream x chunks (resident in SBUF), cast fp32->bf16, accumulate
           G = sum_i xb_i^T @ xb_i into 2 PSUM banks ([128,256] each).
  Softmax: row max/exp/sum on the [C,C] PSUM tiles (fp32), fold beta/Z into
           the rows, transpose 128x128 blocks on the PE to get
           att_s^T = (beta * softmax(G))^T in SBUF (bf16).
  Phase 2: per chunk: PE-transpose the bf16 chunk to get a^T slices, matmul
           psum[n,d] = sum_c a^T[c,n]^T @ att_s^T[c,d], add the fp32 residual
           from the resident x, DMA out.
"""

import sys

import numpy as np

sys.path.insert(0, "/opt/trn_rl_repo")

import concourse.bass as bass  # noqa: E402
import concourse.tile as tile  # noqa: E402
from concourse import bacc, mybir  # noqa: E402
from concourse.masks import make_identity  # noqa: E402

P = 128          # partitions / chunk rows
C = 256          # channels
N = 16384        # H*W rows per sample
NCH = N // P     # 128 row-chunks
DG = 8           # chunks per input DMA (1 MiB per dma_start)
CG = 4           # chunks per gpsimd cast granule
OG = 4           # chunks per output staging granule / DMA
NB = 8           # batch == cores
F32 = mybir.dt.float32
BF16 = mybir.dt.bfloat16


def build_program() -> bass.Bass:
    nc = bacc.Bacc(None, target_bir_lowering=False)
    x = nc.dram_tensor("x", [N, C], F32, kind="ExternalInput")
    beta = nc.dram_tensor("beta", [1, 1], F32, kind="ExternalInput")
    out = nc.dram_tensor("out", [N, C], F32, kind="ExternalOutput")

    # chunk-major views: [p, t, c] = row t*128+p, channel c
    x_v = x.rearrange("(t p) c -> p t c", p=P)
    out_v = out.rearrange("(t p) c -> p t c", p=P)

    with tile.TileContext(nc) as tc:
        with (
            tc.tile_pool(name="singles", bufs=1) as singles,
            tc.tile_pool(name="xres", bufs=1) as xres_pool,
            tc.tile_pool(name="xb", bufs=3) as xb_pool,
            tc.tile_pool(name="att", bufs=1) as att_pool,
            tc.tile_pool(name="stat", bufs=2) as stat_pool,
        ):
            ident = singles.tile([P, P], BF16, tag="ident")
            make_identity(nc, ident)
            beta_sb = singles.tile([P, 1], F32, tag="beta")
            nc.gpsimd.dma_start(out=beta_sb, in_=beta[:].to_broadcast((P, 1)))

            # residents: x (fp32, matmul source + residual) and a^T (bf16)
            x_res = xres_pool.tile([P, NCH, C], F32, tag="xres")
            aT_res = singles.tile([P, NCH, 2, P], BF16, tag="aT")
            # first granules small so compute starts early, rest 1 MiB
            in_granules = [(0, 1), (1, 1), (2, 2), (4, 4)] + [
                (s, DG) for s in range(DG, NCH, DG)
            ]
            for gi, (s, sz) in enumerate(in_granules):
                in_eng = nc.scalar if gi % 2 == 0 else nc.sync
                in_eng.dma_start(
                    out=x_res[:, s : s + sz, :],
                    in_=x_v[:, s : s + sz, :],
                )

            att_t = [
                att_pool.tile([P, C], BF16, tag=f"attT{k}", name=f"attT{k}")
                for k in range(2)
            ]
            with (
                tc.tile_pool(name="gps", bufs=1, space="PSUM") as gps_pool,
                tc.tile_pool(name="tps", bufs=3, space="PSUM") as tps_pool,
            ):
                # ---- Phase 1: G = a^T a (PSUM-accumulated over 128 chunks),
                #      plus on-the-fly PE transposes building a^T resident ----
                G = [
                    gps_pool.tile([P, C], F32, tag=f"G{j}", name=f"G{j}")
                    for j in range(2)
                ]
                cast_granules = [(0, 1), (1, 1), (2, 2)] + [
                    (s, CG) for s in range(CG, NCH, CG)
                ]
                xb_of = {}  # chunk -> (granule tile, offset)
                for s, sz in cast_granules:
                    xbg = xb_pool.tile([P, sz, C], BF16, tag=f"xb{sz}", name=f"xb_{s}")
                    nc.vector.tensor_copy(out=xbg, in_=x_res[:, s : s + sz, :])
                    for ii in range(sz):
                        xb_of[s + ii] = (xbg, ii)
                for t in range(NCH // 2):  # pairs of chunks share one PSUM tile
                    i0 = 2 * t
                    tps = tps_pool.tile([P, 2, 2, P], BF16, tag="tp", name=f"tp_{i0}")
                    for u in range(2):
                        i = i0 + u
                        xbg, off = xb_of[i]
                        xb = xbg[:, off, :]
                        for j in range(2):
                            nc.tensor.matmul(
                                G[j][:],
                                lhsT=xb[:, j * P : (j + 1) * P],
                                rhs=xb[:],
                                start=(i == 0),
                                stop=(i == NCH - 1),
                            )
                        for j in range(2):
                            nc.tensor.transpose(
                                tps[:, u, j, :],
                                xb[:, j * P : (j + 1) * P],
                                ident,
                            )
                    nc.scalar.copy(out=aT_res[:, i0 : i0 + 2, :, :], in_=tps[:])

                # ---- Softmax rows of G (scaled by beta), then transpose ----
                for j in range(2):
                    nmax = stat_pool.tile([P, 1], F32, tag="nmax", name=f"nmax{j}")
                    nc.vector.reduce_max(
                        out=nmax, in_=G[j][:], axis=mybir.AxisListType.X, negate=True
                    )
                    attj = att_pool.tile([P, C], BF16, tag=f"att{j}", name=f"att{j}")
                    zsum = stat_pool.tile([P, 1], F32, tag="zsum", name=f"zsum{j}")
                    nc.scalar.activation(
                        out=attj,
                        in_=G[j][:],
                        func=mybir.ActivationFunctionType.Exp,
                        bias=nmax,
                        scale=1.0,
                        accum_out=zsum,
                    )
                    scl = stat_pool.tile([P, 1], F32, tag="scl", name=f"scl{j}")
                    nc.vector.reciprocal(out=scl, in_=zsum)
                    nc.vector.tensor_mul(out=scl, in0=scl, in1=beta_sb)
                    nc.vector.tensor_scalar_mul(out=attj, in0=attj, scalar1=scl)
                    for k in range(2):
                        tpa = tps_pool.tile(
                            [P, 2, 2, P], BF16, tag="tp", name=f"tpa{j}{k}"
                        )
                        nc.tensor.transpose(
                            tpa[:, 0, 0, :], attj[:, k * P : (k + 1) * P], ident
                        )
                        nc.scalar.copy(
                            out=att_t[k][:, j * P : (j + 1) * P], in_=tpa[:, 0, 0, :]
                        )

            # ---- Phase 2: out = x + a @ att_s^T ----
            # 4 chunks share one 2-bank PSUM tile; residual-added in place into
            # x_res (chunks are dead after their add), then DMA'd out per group
            with tc.tile_pool(name="ops", bufs=3, space="PSUM") as ops_pool:
                for g in range(NCH // OG):
                    i0 = g * OG
                    ops = ops_pool.tile([P, OG, C], F32, tag="op", name=f"op{i0}")
                    for u in range(OG):
                        for j in range(2):
                            nc.tensor.matmul(
                                ops[:, u, :],
                                lhsT=aT_res[:, i0 + u, j, :],
                                rhs=att_t[j][:],
                                start=(j == 0),
                                stop=(j == 1),
                            )
                    nc.vector.tensor_add(
                        out=x_res[:, i0 : i0 + OG, :],
                        in0=ops[:],
                        in1=x_res[:, i0 : i0 + OG, :],
                    )
                    if g % 2 == 1:  # one 1 MiB DMA per two add-groups
                        out_eng = nc.sync if g % 4 == 1 else nc.scalar
                        out_eng.dma_start(
                            out=out_v[:, i0 - OG : i0 + OG, :],
                            in_=x_res[:, i0 - OG : i0 + OG, :],
                        )
    nc.finalize()
    return nc


_NC_CACHE = None


def _get_program() -> bass.Bass:
    global _NC_CACHE
    if _NC_CACHE is None:
        _NC_CACHE = build_program()
    return _NC_CACHE


def run(x: np.ndarray, beta: np.ndarray, trace: bool = False):
    """Shard over 8 cores, run, gather. Returns (out [8,128,128,256], results)."""
    from concourse.bass_utils import run_bass_kernel_spmd

    x = np.ascontiguousarray(np.asarray(x, dtype=np.float32)).reshape(NB, N, C)
    beta_arr = np.ascontiguousarray(
        np.asarray(beta, dtype=np.float32).reshape(1, 1)
    )
    nc = _get_program()
    in_maps = [{"x": x[b], "beta": beta_arr} for b in range(NB)]
    res = run_bass_kernel_spmd(nc, in_maps, core_ids=list(range(NB)), trace=trace)
    outs = np.stack(
        [res.results[b]["out"].reshape(P, P, C) for b in range(NB)], axis=0
    )
    return outs, res


def kernel(**inputs) -> np.ndarray:
    x = np.asarray(inputs["x"])
    beta = np.asarray(inputs["beta"])
    outs, _ = run(x, beta, trace=False)
    return outs.astype(np.float32, copy=False)


# revision 25
# speedup vs baseline: 1.2015x; 1.2015x over previous
"""Channel-attention kernel for Trainium2 (Bass/Tile), 8-core data parallel.

Computes, per batch sample b (x: [B=8, H=128, W=128, C=256] fp32):
    a   = x[b].reshape(N=16384, C)
    G   = a^T @ a                      # [C, C]
    att = softmax(G, axis=-1)
    out = x[b] + beta * (a @ att^T)

Sharding: pure data parallel, one sample per NeuronCore (B == n_cores == 8).

Per-core pipeline (N=16384 rows split into 128 chunks of 128):
  Phase 1: stream x chunks (resident in SBUF), cast fp32->bf16, accumulate
           G = sum_i xb_i^T @ xb_i into 2 PSUM banks ([128,256] each).
  Softmax: row max/exp/sum on the [C,C] PSUM tiles (fp32), fold beta/Z into
           the rows, transpose 128x128 blocks on the PE to get
           att_s^T = (beta * softmax(G))^T in SBUF (bf16).
  Phase 2: per chunk: PE-transpose the bf16 chunk to get a^T slices, matmul
           psum[n,d] = sum_c a^T[c,n]^T @ att_s^T[c,d], add the fp32 residual
           from the resident x, DMA out.
"""

import sys

import numpy as np

sys.path.insert(0, "/opt/trn_rl_repo")

import concourse.bass as bass  # noqa: E402
import concourse.tile as tile  # noqa: E402
from concourse import bacc, mybir  # noqa: E402
from concourse.masks import make_identity  # noqa: E402

P = 128          # partitions / chunk rows
C = 256          # channels
N = 16384        # H*W rows per sample
NCH = N // P     # 128 row-chunks
DG = 8           # chunks per input DMA (1 MiB per dma_start)
CG = 4           # chunks per gpsimd cast granule
OG = 4           # chunks per output staging granule / DMA
NB = 8           # batch == cores
F32 = mybir.dt.float32
BF16 = mybir.dt.bfloat16


def build_program() -> bass.Bass:
    nc = bacc.Bacc(None, target_bir_lowering=False)
    x = nc.dram_tensor("x", [N, C], F32, kind="ExternalInput")
    beta = nc.dram_tensor("beta", [1, 1], F32, kind="ExternalInput")
    out = nc.dram_tensor("out", [N, C], F32, kind="ExternalOutput")

    # chunk-major views: [p, t, c] = row t*128+p, channel c
    x_v = x.rearrange("(t p) c -> p t c", p=P)
    out_v = out.rearrange("(t p) c -> p t c", p=P)

    with tile.TileContext(nc) as tc:
        with (
            tc.tile_pool(name="singles", bufs=1) as singles,
            tc.tile_pool(name="xres", bufs=1) as xres_pool,
            tc.tile_pool(name="xb", bufs=3) as xb_pool,
            tc.tile_pool(name="att", bufs=1) as att_pool,
            tc.tile_pool(name="stat", bufs=2) as stat_pool,
        ):
            ident = singles.tile([P, P], BF16, tag="ident")
            make_identity(nc, ident)
            beta_sb = singles.tile([P, 1], F32, tag="beta")
            nc.gpsimd.dma_start(out=beta_sb, in_=beta[:].to_broadcast((P, 1)))

            # residents: x (fp32, matmul source + residual) and a^T (bf16)
            x_res = xres_pool.tile([P, NCH, C], F32, tag="xres")
            aT_res = singles.tile([P, NCH, 2, P], BF16, tag="aT")
            # first granules small so compute starts early, rest 1 MiB
            in_granules = [(0, 1), (1, 1), (2, 2), (4, 4)] + [
                (s, DG) for s in range(DG, NCH, DG)
            ]
            for s, sz in in_granules:
                nc.sync.dma_start(
                    out=x_res[:, s : s + sz, :],
                    in_=x_v[:, s : s + sz, :],
                )

            att_t = [
                att_pool.tile([P, C], BF16, tag=f"attT{k}", name=f"attT{k}")
                for k in range(2)
            ]
            with (
                tc.tile_pool(name="gps", bufs=1, space="PSUM") as gps_pool,
                tc.tile_pool(name="tps", bufs=3, space="PSUM") as tps_pool,
            ):
                # ---- Phase 1: G = a^T a (PSUM-accumulated over 128 chunks),
                #      plus on-the-fly PE transposes building a^T resident ----
                G = [
                    gps_pool.tile([P, C], F32, tag=f"G{j}", name=f"G{j}")
                    for j in range(2)
                ]
                cast_granules = [(0, 1), (1, 1), (2, 2)] + [
                    (s, CG) for s in range(CG, NCH, CG)
                ]
                xb_of = {}  # chunk -> (granule tile, offset)
                for s, sz in cast_granules:
                    xbg = xb_pool.tile([P, sz, C], BF16, tag=f"xb{sz}", name=f"xb_{s}")
                    nc.vector.tensor_copy(out=xbg, in_=x_res[:, s : s + sz, :])
                    for ii in range(sz):
                        xb_of[s + ii] = (xbg, ii)
                for t in range(NCH // 2):  # pairs of chunks share one PSUM tile
                    i0 = 2 * t
                    tps = tps_pool.tile([P, 2, 2, P], BF16, tag="tp", name=f"tp_{i0}")
                    for u in range(2):
                        i = i0 + u
                        xbg, off = xb_of[i]
                        xb = xbg[:, off, :]
                        for j in range(2):
                            nc.tensor.matmul(
                                G[j][:],
                                lhsT=xb[:, j * P : (j + 1) * P],
                                rhs=xb[:],
                                start=(i == 0),
                                stop=(i == NCH - 1),
                            )
                        for j in range(2):
                            nc.tensor.transpose(
                                tps[:, u, j, :],
                                xb[:, j * P : (j + 1) * P],
                                ident,
                            )
                    nc.scalar.copy(out=aT_res[:, i0 : i0 + 2, :, :], in_=tps[:])

                # ---- Softmax rows of G (scaled by beta), then transpose ----
                for j in range(2):
                    nmax = stat_pool.tile([P, 1], F32, tag="nmax", name=f"nmax{j}")
                    nc.vector.reduce_max(
                        out=nmax, in_=G[j][:], axis=mybir.AxisListType.X, negate=True
                    )
                    attj = att_pool.tile([P, C], BF16, tag=f"att{j}", name=f"att{j}")
                    zsum = stat_pool.tile([P, 1], F32, tag="zsum", name=f"zsum{j}")
                    nc.scalar.activation(
                        out=attj,
                        in_=G[j][:],
                        func=mybir.ActivationFunctionType.Exp,
                        bias=nmax,
                        scale=1.0,
                        accum_out=zsum,
                    )
                    scl = stat_pool.tile([P, 1], F32, tag="scl", name=f"scl{j}")
                    nc.vector.reciprocal(out=scl, in_=zsum)
                    nc.vector.tensor_mul(out=scl, in0=scl, in1=beta_sb)
                    nc.vector.tensor_scalar_mul(out=attj, in0=attj, scalar1=scl)
                    for k in range(2):
                        tpa = tps_pool.tile(
                            [P, 2, 2, P], BF16, tag="tp", name=f"tpa{j}{k}"
                        )
                        nc.tensor.transpose(
                            tpa[:, 0, 0, :], attj[:, k * P : (k + 1) * P], ident
                        )
                        nc.scalar.copy(
                            out=att_t[k][:, j * P : (j + 1) * P], in_=tpa[:, 0, 0, :]
                        )

            # ---- Phase 2: out = x + a @ att_s^T ----
            # 4 chunks share one 2-bank PSUM tile; residual-added in place into
            # x_res (chunks are dead after their add), then DMA'd out per group
            with tc.tile_pool(name="ops", bufs=3, space="PSUM") as ops_pool:
                for g in range(NCH // OG):
                    i0 = g * OG
                    ops = ops_pool.tile([P, OG, C], F32, tag="op", name=f"op{i0}")
                    for u in range(OG):
                        for j in range(2):
                            nc.tensor.matmul(
                                ops[:, u, :],
                                lhsT=aT_res[:, i0 + u, j, :],
                                rhs=att_t[j][:],
                                start=(j == 0),
                                stop=(j == 1),
                            )
                    nc.vector.tensor_add(
                        out=x_res[:, i0 : i0 + OG, :],
                        in0=ops[:],
                        in1=x_res[:, i0 : i0 + OG, :],
                    )
                    if g % 2 == 1:  # one 1 MiB DMA per two add-groups
                        out_eng = nc.sync if g % 4 == 1 else nc.scalar
                        out_eng.dma_start(
                            out=out_v[:, i0 - OG : i0 + OG, :],
                            in_=x_res[:, i0 - OG : i0 + OG, :],
                        )
    nc.finalize()
    return nc


_NC_CACHE = None


def _get_program() -> bass.Bass:
    global _NC_CACHE
    if _NC_CACHE is None:
        _NC_CACHE = build_program()
    return _NC_CACHE


def run(x: np.ndarray, beta: np.ndarray, trace: bool = False):
    """Shard over 8 cores, run, gather. Returns (out [8,128,128,256], results)."""
    from concourse.bass_utils import run_bass_kernel_spmd

    x = np.ascontiguousarray(np.asarray(x, dtype=np.float32)).reshape(NB, N, C)
    beta_arr = np.ascontiguousarray(
        np.asarray(beta, dtype=np.float32).reshape(1, 1)
    )
    nc = _get_program()
    in_maps = [{"x": x[b], "beta": beta_arr} for b in range(NB)]
    res = run_bass_kernel_spmd(nc, in_maps, core_ids=list(range(NB)), trace=trace)
    outs = np.stack(
        [res.results[b]["out"].reshape(P, P, C) for b in range(NB)], axis=0
    )
    return outs, res


def kernel(**inputs) -> np.ndarray:
    x = np.asarray(inputs["x"])
    beta = np.asarray(inputs["beta"])
    outs, _ = run(x, beta, trace=False)
    return outs.astype(np.float32, copy=False)


# revision 26
# speedup vs baseline: 1.2985x; 1.0807x over previous
"""Channel-attention kernel for Trainium2 (Bass/Tile), 8-core data parallel.

Computes, per batch sample b (x: [B=8, H=128, W=128, C=256] fp32):
    a   = x[b].reshape(N=16384, C)
    G   = a^T @ a                      # [C, C]
    att = softmax(G, axis=-1)
    out = x[b] + beta * (a @ att^T)

Sharding: pure data parallel, one sample per NeuronCore (B == n_cores == 8).

Per-core pipeline (N=16384 rows split into 128 chunks of 128):
  Phase 1: stream x chunks (resident in SBUF), cast fp32->bf16, accumulate
           G = sum_i xb_i^T @ xb_i into 2 PSUM banks ([128,256] each).
  Softmax: row max/exp/sum on the [C,C] PSUM tiles (fp32), fold beta/Z into
           the rows, transpose 128x128 blocks on the PE to get
           att_s^T = (beta * softmax(G))^T in SBUF (bf16).
  Phase 2: per chunk: PE-transpose the bf16 chunk to get a^T slices, matmul
           psum[n,d] = sum_c a^T[c,n]^T @ att_s^T[c,d], add the fp32 residual
           from the resident x, DMA out.
"""

import sys

import numpy as np

sys.path.insert(0, "/opt/trn_rl_repo")

import concourse.bass as bass  # noqa: E402
import concourse.tile as tile  # noqa: E402
from concourse import bacc, mybir  # noqa: E402
from concourse.masks import make_identity  # noqa: E402

P = 128          # partitions / chunk rows
C = 256          # channels
N = 16384        # H*W rows per sample
NCH = N // P     # 128 row-chunks
DG = 8           # chunks per input DMA (1 MiB per dma_start)
CG = 4           # chunks per gpsimd cast granule
OG = 4           # chunks per output staging granule / DMA
NB = 8           # batch == cores
F32 = mybir.dt.float32
BF16 = mybir.dt.bfloat16


def build_program() -> bass.Bass:
    nc = bacc.Bacc(None, target_bir_lowering=False)
    x = nc.dram_tensor("x", [N, C], F32, kind="ExternalInput")
    beta = nc.dram_tensor("beta", [1, 1], F32, kind="ExternalInput")
    out = nc.dram_tensor("out", [N, C], F32, kind="ExternalOutput")

    # chunk-major views: [p, t, c] = row t*128+p, channel c
    x_v = x.rearrange("(t p) c -> p t c", p=P)
    out_v = out.rearrange("(t p) c -> p t c", p=P)

    with tile.TileContext(nc) as tc:
        with (
            tc.tile_pool(name="singles", bufs=1) as singles,
            tc.tile_pool(name="xres", bufs=1) as xres_pool,
            tc.tile_pool(name="xb", bufs=3) as xb_pool,
            tc.tile_pool(name="att", bufs=1) as att_pool,
            tc.tile_pool(name="stat", bufs=2) as stat_pool,
        ):
            ident = singles.tile([P, P], BF16, tag="ident")
            make_identity(nc, ident)
            beta_sb = singles.tile([P, 1], F32, tag="beta")
            nc.gpsimd.dma_start(out=beta_sb, in_=beta[:].to_broadcast((P, 1)))

            # residents: x (fp32, matmul source + residual) and a^T (bf16)
            x_res = xres_pool.tile([P, NCH, C], F32, tag="xres")
            aT_res = singles.tile([P, NCH, 2, P], BF16, tag="aT")
            # first granules small so compute starts early, rest 1 MiB
            in_granules = [(0, 1), (1, 1), (2, 2), (4, 4)] + [
                (s, DG) for s in range(DG, NCH, DG)
            ]
            for s, sz in in_granules:
                nc.sync.dma_start(
                    out=x_res[:, s : s + sz, :],
                    in_=x_v[:, s : s + sz, :],
                )

            att_t = [
                att_pool.tile([P, C], BF16, tag=f"attT{k}", name=f"attT{k}")
                for k in range(2)
            ]
            with (
                tc.tile_pool(name="gps", bufs=1, space="PSUM") as gps_pool,
                tc.tile_pool(name="tps", bufs=3, space="PSUM") as tps_pool,
            ):
                # ---- Phase 1: G = a^T a (PSUM-accumulated over 128 chunks),
                #      plus on-the-fly PE transposes building a^T resident ----
                G = [
                    gps_pool.tile([P, C], F32, tag=f"G{j}", name=f"G{j}")
                    for j in range(2)
                ]
                cast_granules = [(0, 1), (1, 1), (2, 2)] + [
                    (s, CG) for s in range(CG, NCH, CG)
                ]
                xb_of = {}  # chunk -> (granule tile, offset)
                for s, sz in cast_granules:
                    xbg = xb_pool.tile([P, sz, C], BF16, tag=f"xb{sz}", name=f"xb_{s}")
                    nc.vector.tensor_copy(out=xbg, in_=x_res[:, s : s + sz, :])
                    for ii in range(sz):
                        xb_of[s + ii] = (xbg, ii)
                for t in range(NCH // 2):  # pairs of chunks share one PSUM tile
                    i0 = 2 * t
                    tps = tps_pool.tile([P, 2, 2, P], BF16, tag="tp", name=f"tp_{i0}")
                    for u in range(2):
                        i = i0 + u
                        xbg, off = xb_of[i]
                        xb = xbg[:, off, :]
                        for j in range(2):
                            nc.tensor.matmul(
                                G[j][:],
                                lhsT=xb[:, j * P : (j + 1) * P],
                                rhs=xb[:],
                                start=(i == 0),
                                stop=(i == NCH - 1),
                            )
                        for j in range(2):
                            nc.tensor.transpose(
                                tps[:, u, j, :],
                                xb[:, j * P : (j + 1) * P],
                                ident,
                            )
                    nc.scalar.copy(out=aT_res[:, i0 : i0 + 2, :, :], in_=tps[:])

                # ---- Softmax rows of G (scaled by beta), then transpose ----
                for j in range(2):
                    nmax = stat_pool.tile([P, 1], F32, tag="nmax", name=f"nmax{j}")
                    nc.vector.reduce_max(
                        out=nmax, in_=G[j][:], axis=mybir.AxisListType.X, negate=True
                    )
                    attj = att_pool.tile([P, C], BF16, tag=f"att{j}", name=f"att{j}")
                    zsum = stat_pool.tile([P, 1], F32, tag="zsum", name=f"zsum{j}")
                    nc.scalar.activation(
                        out=attj,
                        in_=G[j][:],
                        func=mybir.ActivationFunctionType.Exp,
                        bias=nmax,
                        scale=1.0,
                        accum_out=zsum,
                    )
                    scl = stat_pool.tile([P, 1], F32, tag="scl", name=f"scl{j}")
                    nc.vector.reciprocal(out=scl, in_=zsum)
                    nc.vector.tensor_mul(out=scl, in0=scl, in1=beta_sb)
                    nc.vector.tensor_scalar_mul(out=attj, in0=attj, scalar1=scl)
                    for k in range(2):
                        tpa = tps_pool.tile(
                            [P, 2, 2, P], BF16, tag="tp", name=f"tpa{j}{k}"
                        )
                        nc.tensor.transpose(
                            tpa[:, 0, 0, :], attj[:, k * P : (k + 1) * P], ident
                        )
                        nc.scalar.copy(
                            out=att_t[k][:, j * P : (j + 1) * P], in_=tpa[:, 0, 0, :]
                        )

            # ---- Phase 2: out = x + a @ att_s^T ----
            # 4 chunks share one 2-bank PSUM tile; residual-added in place into
            # x_res (chunks are dead after their add), then DMA'd out per group
            with tc.tile_pool(name="ops", bufs=3, space="PSUM") as ops_pool:
                for g in range(NCH // OG):
                    i0 = g * OG
                    ops = ops_pool.tile([P, OG, C], F32, tag="op", name=f"op{i0}")
                    for u in range(OG):
                        for j in range(2):
                            nc.tensor.matmul(
                                ops[:, u, :],
                                lhsT=aT_res[:, i0 + u, j, :],
                                rhs=att_t[j][:],
                                start=(j == 0),
                                stop=(j == 1),
                            )
                    nc.vector.tensor_add(
                        out=x_res[:, i0 : i0 + OG, :],
                        in0=ops[:],
                        in1=x_res[:, i0 : i0 + OG, :],
                    )
                    out_eng = nc.sync if g % 2 == 0 else nc.scalar
                    out_eng.dma_start(
                        out=out_v[:, i0 : i0 + OG, :],
                        in_=x_res[:, i0 : i0 + OG, :],
                    )
    nc.finalize()
    return nc


_NC_CACHE = None


def _get_program() -> bass.Bass:
    global _NC_CACHE
    if _NC_CACHE is None:
        _NC_CACHE = build_program()
    return _NC_CACHE


def run(x: np.ndarray, beta: np.ndarray, trace: bool = False):
    """Shard over 8 cores, run, gather. Returns (out [8,128,128,256], results)."""
    from concourse.bass_utils import run_bass_kernel_spmd

    x = np.ascontiguousarray(np.asarray(x, dtype=np.float32)).reshape(NB, N, C)
    beta_arr = np.ascontiguousarray(
        np.asarray(beta, dtype=np.float32).reshape(1, 1)
    )
    nc = _get_program()
    in_maps = [{"x": x[b], "beta": beta_arr} for b in range(NB)]
    res = run_bass_kernel_spmd(nc, in_maps, core_ids=list(range(NB)), trace=trace)
    outs = np.stack(
        [res.results[b]["out"].reshape(P, P, C) for b in range(NB)], axis=0
    )
    return outs, res


def kernel(**inputs) -> np.ndarray:
    x = np.asarray(inputs["x"])
    beta = np.asarray(inputs["beta"])
    outs, _ = run(x, beta, trace=False)
    return outs.astype(np.float32, copy=False)


# revision 27
# speedup vs baseline: 1.3509x; 1.0403x over previous
"""Channel-attention kernel for Trainium2 (Bass/Tile), 8-core data parallel.

Computes, per batch sample b (x: [B=8, H=128, W=128, C=256] fp32):
    a   = x[b].reshape(N=16384, C)
    G   = a^T @ a                      # [C, C]
    att = softmax(G, axis=-1)
    out = x[b] + beta * (a @ att^T)

Sharding: pure data parallel, one sample per NeuronCore (B == n_cores == 8).

Per-core pipeline (N=16384 rows split into 128 chunks of 128):
  Phase 1: stream x chunks (resident in SBUF), cast fp32->bf16, accumulate
           G = sum_i xb_i^T @ xb_i into 2 PSUM banks ([128,256] each).
  Softmax: row max/exp/sum on the [C,C] PSUM tiles (fp32), fold beta/Z into
           the rows, transpose 128x128 blocks on the PE to get
           att_s^T = (beta * softmax(G))^T in SBUF (bf16).
  Phase 2: per chunk: PE-transpose the bf16 chunk to get a^T slices, matmul
           psum[n,d] = sum_c a^T[c,n]^T @ att_s^T[c,d], add the fp32 residual
           from the resident x, DMA out.
"""

import sys

import numpy as np

sys.path.insert(0, "/opt/trn_rl_repo")

import concourse.bass as bass  # noqa: E402
import concourse.tile as tile  # noqa: E402
from concourse import bacc, mybir  # noqa: E402
from concourse.masks import make_identity  # noqa: E402

P = 128          # partitions / chunk rows
C = 256          # channels
N = 16384        # H*W rows per sample
NCH = N // P     # 128 row-chunks
DG = 8           # chunks per input DMA (1 MiB per dma_start)
CG = 4           # chunks per gpsimd cast granule
OG = 4           # chunks per output staging granule / DMA
NB = 8           # batch == cores
F32 = mybir.dt.float32
BF16 = mybir.dt.bfloat16


def build_program() -> bass.Bass:
    nc = bacc.Bacc(None, target_bir_lowering=False)
    x = nc.dram_tensor("x", [N, C], F32, kind="ExternalInput")
    beta = nc.dram_tensor("beta", [1, 1], F32, kind="ExternalInput")
    out = nc.dram_tensor("out", [N, C], F32, kind="ExternalOutput")

    # partition-contiguous views: [p, t, c] = row p*NCH+t, channel c.
    # Each partition covers a contiguous 128 KiB HBM span, so every DMA is
    # one long contiguous run per partition (minimal descriptors). All stages
    # use this same row permutation consistently; G sums over all rows, so
    # the permutation does not change the result.
    x_v = x.rearrange("(p t) c -> p t c", p=P)
    out_v = out.rearrange("(p t) c -> p t c", p=P)

    with tile.TileContext(nc) as tc:
        with (
            tc.tile_pool(name="singles", bufs=1) as singles,
            tc.tile_pool(name="xres", bufs=1) as xres_pool,
            tc.tile_pool(name="xb", bufs=3) as xb_pool,
            tc.tile_pool(name="att", bufs=1) as att_pool,
            tc.tile_pool(name="stat", bufs=2) as stat_pool,
        ):
            ident = singles.tile([P, P], BF16, tag="ident")
            make_identity(nc, ident)
            beta_sb = singles.tile([P, 1], F32, tag="beta")
            nc.gpsimd.dma_start(out=beta_sb, in_=beta[:].to_broadcast((P, 1)))

            # residents: x (fp32, matmul source + residual) and a^T (bf16)
            x_res = xres_pool.tile([P, NCH, C], F32, tag="xres")
            aT_res = singles.tile([P, NCH, 2, P], BF16, tag="aT")
            # first granules small so compute starts early, rest 1 MiB
            in_granules = [(0, 1), (1, 1), (2, 2), (4, 4)] + [
                (s, DG) for s in range(DG, NCH, DG)
            ]
            for s, sz in in_granules:
                nc.sync.dma_start(
                    out=x_res[:, s : s + sz, :],
                    in_=x_v[:, s : s + sz, :],
                )

            att_t = [
                att_pool.tile([P, C], BF16, tag=f"attT{k}", name=f"attT{k}")
                for k in range(2)
            ]
            with (
                tc.tile_pool(name="gps", bufs=1, space="PSUM") as gps_pool,
                tc.tile_pool(name="tps", bufs=3, space="PSUM") as tps_pool,
            ):
                # ---- Phase 1: G = a^T a (PSUM-accumulated over 128 chunks),
                #      plus on-the-fly PE transposes building a^T resident ----
                G = [
                    gps_pool.tile([P, C], F32, tag=f"G{j}", name=f"G{j}")
                    for j in range(2)
                ]
                cast_granules = [(0, 1), (1, 1), (2, 2)] + [
                    (s, CG) for s in range(CG, NCH, CG)
                ]
                xb_of = {}  # chunk -> (granule tile, offset)
                for s, sz in cast_granules:
                    xbg = xb_pool.tile([P, sz, C], BF16, tag=f"xb{sz}", name=f"xb_{s}")
                    nc.vector.tensor_copy(out=xbg, in_=x_res[:, s : s + sz, :])
                    for ii in range(sz):
                        xb_of[s + ii] = (xbg, ii)
                for t in range(NCH // 2):  # pairs of chunks share one PSUM tile
                    i0 = 2 * t
                    tps = tps_pool.tile([P, 2, 2, P], BF16, tag="tp", name=f"tp_{i0}")
                    for u in range(2):
                        i = i0 + u
                        xbg, off = xb_of[i]
                        xb = xbg[:, off, :]
                        for j in range(2):
                            nc.tensor.matmul(
                                G[j][:],
                                lhsT=xb[:, j * P : (j + 1) * P],
                                rhs=xb[:],
                                start=(i == 0),
                                stop=(i == NCH - 1),
                            )
                        for j in range(2):
                            nc.tensor.transpose(
                                tps[:, u, j, :],
                                xb[:, j * P : (j + 1) * P],
                                ident,
                            )
                    nc.scalar.copy(out=aT_res[:, i0 : i0 + 2, :, :], in_=tps[:])

                # ---- Softmax rows of G (scaled by beta), then transpose ----
                for j in range(2):
                    nmax = stat_pool.tile([P, 1], F32, tag="nmax", name=f"nmax{j}")
                    nc.vector.reduce_max(
                        out=nmax, in_=G[j][:], axis=mybir.AxisListType.X, negate=True
                    )
                    attj = att_pool.tile([P, C], BF16, tag=f"att{j}", name=f"att{j}")
                    zsum = stat_pool.tile([P, 1], F32, tag="zsum", name=f"zsum{j}")
                    nc.scalar.activation(
                        out=attj,
                        in_=G[j][:],
                        func=mybir.ActivationFunctionType.Exp,
                        bias=nmax,
                        scale=1.0,
                        accum_out=zsum,
                    )
                    scl = stat_pool.tile([P, 1], F32, tag="scl", name=f"scl{j}")
                    nc.vector.reciprocal(out=scl, in_=zsum)
                    nc.vector.tensor_mul(out=scl, in0=scl, in1=beta_sb)
                    nc.vector.tensor_scalar_mul(out=attj, in0=attj, scalar1=scl)
                    for k in range(2):
                        tpa = tps_pool.tile(
                            [P, 2, 2, P], BF16, tag="tp", name=f"tpa{j}{k}"
                        )
                        nc.tensor.transpose(
                            tpa[:, 0, 0, :], attj[:, k * P : (k + 1) * P], ident
                        )
                        nc.scalar.copy(
                            out=att_t[k][:, j * P : (j + 1) * P], in_=tpa[:, 0, 0, :]
                        )

            # ---- Phase 2: out = x + a @ att_s^T ----
            # 4 chunks share one 2-bank PSUM tile; residual-added in place into
            # x_res (chunks are dead after their add), then DMA'd out per group
            with tc.tile_pool(name="ops", bufs=3, space="PSUM") as ops_pool:
                for g in range(NCH // OG):
                    i0 = g * OG
                    ops = ops_pool.tile([P, OG, C], F32, tag="op", name=f"op{i0}")
                    for u in range(OG):
                        for j in range(2):
                            nc.tensor.matmul(
                                ops[:, u, :],
                                lhsT=aT_res[:, i0 + u, j, :],
                                rhs=att_t[j][:],
                                start=(j == 0),
                                stop=(j == 1),
                            )
                    nc.vector.tensor_add(
                        out=x_res[:, i0 : i0 + OG, :],
                        in0=ops[:],
                        in1=x_res[:, i0 : i0 + OG, :],
                    )
                    out_eng = nc.sync if g % 2 == 0 else nc.scalar
                    out_eng.dma_start(
                        out=out_v[:, i0 : i0 + OG, :],
                        in_=x_res[:, i0 : i0 + OG, :],
                    )
    nc.finalize()
    return nc


_NC_CACHE = None


def _get_program() -> bass.Bass:
    global _NC_CACHE
    if _NC_CACHE is None:
        _NC_CACHE = build_program()
    return _NC_CACHE


def run(x: np.ndarray, beta: np.ndarray, trace: bool = False):
    """Shard over 8 cores, run, gather. Returns (out [8,128,128,256], results)."""
    from concourse.bass_utils import run_bass_kernel_spmd

    x = np.ascontiguousarray(np.asarray(x, dtype=np.float32)).reshape(NB, N, C)
    beta_arr = np.ascontiguousarray(
        np.asarray(beta, dtype=np.float32).reshape(1, 1)
    )
    nc = _get_program()
    in_maps = [{"x": x[b], "beta": beta_arr} for b in range(NB)]
    res = run_bass_kernel_spmd(nc, in_maps, core_ids=list(range(NB)), trace=trace)
    outs = np.stack(
        [res.results[b]["out"].reshape(P, P, C) for b in range(NB)], axis=0
    )
    return outs, res


def kernel(**inputs) -> np.ndarray:
    x = np.asarray(inputs["x"])
    beta = np.asarray(inputs["beta"])
    outs, _ = run(x, beta, trace=False)
    return outs.astype(np.float32, copy=False)
